# revision 1
# baseline (speedup 1.0000x reference)
"""Deformable (region-aware) matmul for Trainium2, data-parallel over batch.

out[b,o,h,w] = sum_r sum_c mat0[b,c,h,w] * mat1[o,c,r] * Alpha[r] * mask[r,h,w]

Shapes: B=8, C=256, H=W=64, O=256, R=8.  One batch per NeuronCore (8 cores).

Per-core algorithm: fold the (region, channel) pair into one contraction axis
K = R*C = 2048 (16 partition tiles of 128).  K-tile t <-> (r = t//2, half =
t%2).  The activation tile for t is X_t[k, p] = mat0[half*128+k, p] *
mask[r, p], produced on the Vector engine as a bf16 multiply (2x mode)
against a host-prebroadcast mask.  Weights W_t[k, o] = mat1[o, half*128+k, r]
* Alpha[r] are host-transposed to lhsT layout.  The Tensor engine accumulates
out[o, p] = sum_t W_t.T @ X_t in PSUM over the 16 K-tiles — one dense matmul
chain (PE-bound by design).  A few zero-input warmup matmuls run during the
input-DMA prologue to lift the PE HAM clock gate before the real stream, and
all input DMAs are chunked so the stream starts as early as possible.
"""

import numpy as np
import ml_dtypes

B, C, H, W_ = 8, 256, 64, 64
O, R = 256, 8
P = H * W_            # 4096 pixels
KT = 2 * R            # 16 K-tiles of 128
PCHUNK = 1024         # pixel chunk per pipeline step
NCHUNK = P // PCHUNK  # 4
MMN = 512             # moving free dim per matmul (one PSUM bank of fp32)
NWARM = 16            # warmup matmuls

_CACHE = {}


def _build():
    import concourse.bacc as bacc
    import concourse.tile as tile
    import concourse.mybir as mybir

    bf16 = mybir.dt.bfloat16
    f32 = mybir.dt.float32

    nc = bacc.Bacc(
        "TRN2",
        target_bir_lowering=False,
        debug=False,
        enable_asserts=False,
        num_devices=8,
    )
    # Per-core inputs (host-prepped layouts, see kernel()):
    #   x[k, half, p]  = mat0[b, half*128+k, p]            (bf16)
    #   w[k, t, o]     = mat1[o, c(t,k), r(t)] * Alpha     (bf16, lhsT layout)
    #   mb[r, q, p]    = mask[r, p] for all q              (bf16, row-broadcast)
    x_d = nc.dram_tensor("x", [128, 2, P], bf16, kind="ExternalInput")
    w_d = nc.dram_tensor("w", [128, KT, O], bf16, kind="ExternalInput")
    mb_d = nc.dram_tensor("mb", [R, 128, P], bf16, kind="ExternalInput")
    y_d = nc.dram_tensor("y", [2, 128, P], f32, kind="ExternalOutput")

    with tile.TileContext(nc) as tc:
        with (
            tc.tile_pool(name="const", bufs=1) as cpool,
            tc.tile_pool(name="xcp", bufs=2) as xcpool,
            tc.tile_pool(name="mbp", bufs=2) as mbpool,
            tc.tile_pool(name="xp", bufs=2) as xpool,
            tc.tile_pool(name="psp", bufs=8, space="PSUM") as pspool,
            tc.tile_pool(name="yp", bufs=4) as ypool,
        ):
            # --- PE warmup: matmuls with no input deps at all (operands are
            # uninitialized SBUF, output PSUM is never read); they run on the
            # Tensor engine while the prologue DMAs are still in flight and
            # lift the HAM clock gate from 1.2 to 2.4 GHz.
            warm_w = cpool.tile([128, 128], bf16, tag="ww")
            warm_x = cpool.tile([128, MMN], bf16, tag="wx")
            nc.vector.memset(warm_w[:], 0.0)
            nc.vector.memset(warm_x[:], 0.0)
            warm_ps = pspool.tile([128, MMN], f32, tag="ps")
            for i in range(NWARM):
                nc.tensor.matmul(
                    warm_ps[:], warm_w[:], warm_x[:], start=True, stop=True
                )

            w_sb = cpool.tile([128, KT, O], bf16, tag="w")

            def dma_w(tq):  # one DMA of 4 K-tiles of weights
                nc.scalar.dma_start(
                    out=w_sb[:, 4 * tq : 4 * (tq + 1), :],
                    in_=w_d[:, 4 * tq : 4 * (tq + 1), :],
                )

            for ci in range(NCHUNK):
                sl = slice(ci * PCHUNK, (ci + 1) * PCHUNK)
                x_sb = xcpool.tile([128, 2, PCHUNK], bf16, tag="xc")
                nc.scalar.dma_start(out=x_sb[:], in_=x_d[:, :, sl])
                mb_sb = mbpool.tile([128, R, PCHUNK], bf16, tag="mb")
                for rq in range(4):  # 4 DMAs of 2 mask rows each
                    nc.sync.dma_start(
                        out=mb_sb[:, 2 * rq : 2 * (rq + 1), :],
                        in_=mb_d[2 * rq : 2 * (rq + 1), :, sl].rearrange(
                            "r q p -> q r p"
                        ),
                    )
                    if ci == 0:
                        # interleave the (resident) weight DMAs behind the
                        # first mask rows so the first X tiles arrive earliest
                        dma_w(rq)
                xt = xpool.tile([128, KT, PCHUNK], bf16, tag="xt")
                for t in range(KT):
                    r, half = t // 2, t % 2
                    nc.vector.tensor_mul(
                        xt[:, t, :], x_sb[:, half, :], mb_sb[:, r, :]
                    )
                for m in range(2):
                    for nn in range(PCHUNK // MMN):
                        nsl = slice(nn * MMN, (nn + 1) * MMN)
                        ps = pspool.tile([128, MMN], f32, tag="ps")
                        for t in range(KT):
                            nc.tensor.matmul(
                                ps[:],
                                w_sb[:, t, m * 128 : (m + 1) * 128],
                                xt[:, t, nsl],
                                start=(t == 0),
                                stop=(t == KT - 1),
                            )
                        y_sb = ypool.tile([128, MMN], f32, tag="y")
                        nc.scalar.copy(y_sb[:], ps[:])
                        st = ci * PCHUNK + nn * MMN
                        nc.sync.dma_start(
                            out=y_d[m, :, st : st + MMN], in_=y_sb[:]
                        )

    nc.compile()
    return nc


def _prep_inputs(mat0, mat1, mask, Alpha, use_alpha):
    bf = ml_dtypes.bfloat16
    m1 = mat1 * np.asarray(Alpha)[None, None, :] if int(use_alpha) else mat1
    # w[k, t, o] with t = r*2 + half, c = half*128 + k
    w = np.transpose(m1.reshape(O, 2, 128, R), (2, 3, 1, 0))  # [k, r, half, o]
    w_h = np.ascontiguousarray(w.reshape(128, KT, O)).astype(bf)
    # mb[r, q, p] = mask[r, p]
    mb_h = np.ascontiguousarray(
        np.broadcast_to(mask.reshape(R, 1, P), (R, 128, P))
    ).astype(bf)
    # x[b][k, half, p] = mat0[b, half*128+k, p]
    x_h = np.ascontiguousarray(
        np.transpose(mat0.reshape(B, 2, 128, P), (0, 2, 1, 3))
    ).astype(bf)
    return x_h, w_h, mb_h


def kernel(mat0, mat1, mask, Alpha, use_alpha, beta):
    from concourse import bass_utils

    mat0 = np.asarray(mat0, dtype=np.float32)
    mat1 = np.asarray(mat1, dtype=np.float32)
    mask = np.asarray(mask, dtype=np.float32)
    Alpha = np.asarray(Alpha, dtype=np.float32)

    if "nc" not in _CACHE:
        _CACHE["nc"] = _build()
    nc = _CACHE["nc"]

    x_h, w_h, mb_h = _prep_inputs(mat0, mat1, mask, Alpha, use_alpha)
    in_maps = [{"x": x_h[b], "w": w_h, "mb": mb_h} for b in range(B)]
    res = bass_utils.run_bass_kernel_spmd(nc, in_maps, core_ids=list(range(B)))
    _CACHE["last_res"] = res
    out = np.stack(
        [res.results[b]["y"].reshape(O, H, W_).astype(np.float32) for b in range(B)]
    )
    return out



# revision 2
# speedup vs baseline: 1.0046x; 1.0046x over previous
"""Deformable (region-aware) matmul for Trainium2, data-parallel over batch.

out[b,o,h,w] = sum_r sum_c mat0[b,c,h,w] * mat1[o,c,r] * Alpha[r] * mask[r,h,w]

Shapes: B=8, C=256, H=W=64, O=256, R=8.  One batch per NeuronCore (8 cores).

Per-core algorithm: fold (region, channel-half) into one contraction axis of
KT = 16 k-tiles of 128.  K-tile t <-> (r = t//4? no: r = t//2, half = t%2).
Activation tile X_t[k, p] = mat0[half*128+k, p] * mask[r, p] is produced on
the Vector engine (bf16 2x mode, broadcast access patterns - the mask is
host-prebroadcast across the 128 partitions).  Weights W_t[k, o] =
mat1[o, half*128+k, r] * Alpha[r] in lhsT layout.  The Tensor engine
accumulates out[o, p] = sum_t W_t.T @ X_t in PSUM - a dense back-to-back
matmul chain at the bf16 roofline (~213 ns per 512-col matmul).

v2 vs the original baseline (82 us): the pixel axis is cut into 10 slabs
(256,256,512*6,256,256) laid out slab-major in HBM so every DMA is
contiguous per partition; weights are DMAed FIRST (the v1 first-matmul
stalled ~9 us on weights queued behind a slow x-chunk); lead-in slabs use
split mask DMAs + split muls so the MM stream starts ~3 us after the fixed
NEFF preamble; the output is stored bf16 (halves the tail DMA) and the last
slab is small so the final DMA drains quickly.  PE warmup matmuls cover the
prologue so the HAM clock gate is released (2.4 GHz) when the stream starts.
"""

import numpy as np
import ml_dtypes

B, C, H, W_ = 8, 256, 64, 64
O, R = 256, 8
P = H * W_            # 4096 pixels
KT = 2 * R            # 16 k-tiles of 128
SL = [256, 256, 512, 512, 512, 512, 512, 512, 256, 256]   # slab pixel counts
assert sum(SL) == P
SOFF = [0] * len(SL)
for _i in range(1, len(SL)):
    SOFF[_i] = SOFF[_i - 1] + SL[_i - 1]
NWARM = 5             # PE warmup matmuls (cover the ~3 us prologue)
MAXPX = max(SL)

_CACHE = {}


def _build():
    import concourse.bacc as bacc
    import concourse.tile as tile
    import concourse.mybir as mybir

    bf16 = mybir.dt.bfloat16
    f32 = mybir.dt.float32

    nc = bacc.Bacc(
        "TRN2",
        target_bir_lowering=False,
        debug=False,
        enable_asserts=False,
        num_devices=8,
    )
    # Per-core inputs, slab-major so every per-slab DMA is contiguous per
    # partition (see _prep_inputs):
    #   xz[k, 2*soff + (half*px + i)]       = mat0[b, half*128+k, soff+i]
    #   mbz[k, 8*soff + (r*px + i)]         = mask[r, soff+i]    (all k)
    #   w[k, t*256 + o]                     = mat1[o, c(t,k), r(t)] * Alpha
    #   yz[k, 2*soff + (m*px + i)]          = out[m*128+k, soff+i]  (bf16)
    xz_d = nc.dram_tensor("xz", [128, 2 * P], bf16, kind="ExternalInput")
    mbz_d = nc.dram_tensor("mbz", [128, 8 * P], bf16, kind="ExternalInput")
    w_d = nc.dram_tensor("w", [128, KT * O], bf16, kind="ExternalInput")
    y_d = nc.dram_tensor("yz", [128, 2 * P], bf16, kind="ExternalOutput")

    with tile.TileContext(nc) as tc:
        with (
            tc.tile_pool(name="const", bufs=1) as cpool,
            tc.tile_pool(name="xp", bufs=4) as xpool,
            tc.tile_pool(name="mbp", bufs=4) as mbpool,
            tc.tile_pool(name="xtp", bufs=3) as xtpool,
            tc.tile_pool(name="psp", bufs=8, space="PSUM") as pspool,
            tc.tile_pool(name="yp", bufs=3) as ypool,
        ):
            w_sb = cpool.tile([128, KT * O], bf16, tag="w")
            # Weights first on the scalar ring, split so w[t] lands just
            # ahead of the matmul chain consuming it.
            wsplit = [(0, 2), (2, 6), (6, 11), (11, 16)]
            for lo, hi in wsplit:
                nc.scalar.dma_start(
                    out=w_sb[:, lo * O : hi * O], in_=w_d[:, lo * O : hi * O]
                )

            # PE warmup: matmuls on zeroed const tiles (gpsimd memsets keep
            # the Vector engine free); output PSUM never read.
            warm_w = cpool.tile([128, 128], bf16, tag="ww")
            warm_x = cpool.tile([128, 512], bf16, tag="wx")
            nc.gpsimd.memset(warm_w[:], 0.0)
            nc.gpsimd.memset(warm_x[:], 0.0)
            warm_ps = pspool.tile([128, 512], f32, tag="ps")
            for _ in range(NWARM):
                nc.tensor.matmul(
                    warm_ps[:], warm_w[:], warm_x[:], start=True, stop=True
                )

            for s, px in enumerate(SL):
                xo, mo, yo = 2 * SOFF[s], 8 * SOFF[s], 2 * SOFF[s]
                x_sb = xpool.tile([128, 2 * MAXPX], bf16, tag="x")
                nc.sync.dma_start(
                    out=x_sb[:, : 2 * px], in_=xz_d[:, xo : xo + 2 * px]
                )
                mb_sb = mbpool.tile([128, 8 * MAXPX], bf16, tag="mb")
                # lead-in slabs: finer mask DMA + mul granularity so the
                # first matmul chain starts as early as possible
                if s == 0:
                    mparts = [(0, 2), (2, 4), (4, 8)]
                    gparts = [(0, 2), (2, 4), (4, 6), (6, 8)]
                elif s == 1:
                    mparts = [(0, 4), (4, 8)]
                    gparts = [(0, 4), (4, 8)]
                else:
                    mparts = [(0, 8)]
                    gparts = [(0, 4), (4, 8)]
                for lo, hi in mparts:
                    nc.sync.dma_start(
                        out=mb_sb[:, lo * px : hi * px],
                        in_=mbz_d[:, mo + lo * px : mo + hi * px],
                    )
                xt = xtpool.tile([128, KT * MAXPX], bf16, tag="xt")
                for lo, hi in gparts:  # rows lo..hi-1, both halves
                    nr = hi - lo
                    out_ap = (
                        xt[:, 2 * lo * px : 2 * hi * px]
                        .rearrange("q (r h f) -> q r h f", r=nr, h=2)
                    )
                    in0 = (
                        x_sb[:, : 2 * px]
                        .rearrange("q (h f) -> q h f", h=2)
                        .unsqueeze(1)
                        .broadcast_to([128, nr, 2, px])
                    )
                    in1 = (
                        mb_sb[:, lo * px : hi * px]
                        .rearrange("q (r f) -> q r f", r=nr)
                        .unsqueeze(2)
                        .broadcast_to([128, nr, 2, px])
                    )
                    nc.vector.tensor_mul(out_ap, in0, in1)
                y_sb = ypool.tile([128, 2 * MAXPX], bf16, tag="y")
                for m in range(2):
                    ps = pspool.tile([128, 512], f32, tag="ps")
                    for t in range(KT):
                        nc.tensor.matmul(
                            ps[:, :px],
                            w_sb[:, t * O + m * 128 : t * O + (m + 1) * 128],
                            xt[:, t * px : (t + 1) * px],
                            start=(t == 0),
                            stop=(t == KT - 1),
                        )
                    nc.scalar.copy(y_sb[:, m * px : (m + 1) * px], ps[:, :px])
                nc.scalar.dma_start(
                    out=y_d[:, yo : yo + 2 * px], in_=y_sb[:, : 2 * px]
                )

    nc.compile()
    return nc


def _prep_inputs(mat0, mat1, mask, Alpha, use_alpha):
    bf = ml_dtypes.bfloat16
    m1 = mat1 * np.asarray(Alpha)[None, None, :] if int(use_alpha) else mat1
    # w[k, t*O + o] with t = r*2 + half, c = half*128 + k
    w = np.transpose(m1.reshape(O, 2, 128, R), (2, 3, 1, 0))  # [k, r, half, o]
    w_h = np.ascontiguousarray(w.reshape(128, KT * O)).astype(bf)

    mask2 = np.asarray(mask, dtype=np.float32).reshape(R, P)
    xr = np.asarray(mat0, dtype=np.float32).reshape(B, 2, 128, P)

    xz = np.empty((B, 128, 2 * P), dtype=bf)
    mbz = np.empty((128, 8 * P), dtype=bf)
    for s, px in enumerate(SL):
        p0 = SOFF[s]
        # xz[b, k, 2*p0 + half*px + i] = mat0[b, half*128+k, p0+i]
        blk = np.transpose(xr[:, :, :, p0 : p0 + px], (0, 2, 1, 3))  # b,k,h,i
        xz[:, :, 2 * p0 : 2 * (p0 + px)] = blk.reshape(B, 128, 2 * px).astype(bf)
        mblk = np.broadcast_to(
            mask2[None, :, p0 : p0 + px], (128, R, px)
        ).reshape(128, R * px)
        mbz[:, 8 * p0 : 8 * (p0 + px)] = mblk.astype(bf)
    return xz, w_h, mbz


def _decode_y(yz):
    # yz [128, 2*P] bf16 slab-major -> out [O, P] float32
    out = np.empty((O, P), dtype=np.float32)
    y = np.asarray(yz)
    for s, px in enumerate(SL):
        p0 = SOFF[s]
        blk = y[:, 2 * p0 : 2 * (p0 + px)].reshape(128, 2, px).astype(np.float32)
        out[:128, p0 : p0 + px] = blk[:, 0, :]
        out[128:, p0 : p0 + px] = blk[:, 1, :]
    return out


def _make_in_maps(mat0, mat1, mask, Alpha, use_alpha):
    xz, w_h, mbz = _prep_inputs(mat0, mat1, mask, Alpha, use_alpha)
    return [{"xz": xz[b], "w": w_h, "mbz": mbz} for b in range(B)]


def kernel(mat0, mat1, mask, Alpha, use_alpha, beta):
    from concourse import bass_utils

    mat0 = np.asarray(mat0, dtype=np.float32)
    mat1 = np.asarray(mat1, dtype=np.float32)
    mask = np.asarray(mask, dtype=np.float32)
    Alpha = np.asarray(Alpha, dtype=np.float32)

    if "nc" not in _CACHE:
        _CACHE["nc"] = _build()
    nc = _CACHE["nc"]

    in_maps = _make_in_maps(mat0, mat1, mask, Alpha, use_alpha)
    res = bass_utils.run_bass_kernel_spmd(nc, in_maps, core_ids=list(range(B)))
    _CACHE["last_res"] = res
    out = np.stack(
        [_decode_y(res.results[b]["yz"]).reshape(O, H, W_) for b in range(B)]
    )
    return out
